# revision 33
# baseline (speedup 1.0000x reference)
"""Causal dot-product attention (low-rank V) on 8 Trainium2 NeuronCores.

Problem: inputs [B=4, N=4096, E=1024], Wq/Wk/Wvdown [E, D=256], Wvup [D, E].
    Q = x Wq; K = x Wk; S = Q K^T / sqrt(D) (causal); A = softmax(S)
    V = x Wvdown Wvup; out = A V

Key algebraic win vs the naive schedule: V has rank <= D=256, so instead of
out = A V with V = x Wvd Wvu (an N x N x E contraction), compute
O_d = A (x Wvd) (N x N x D) and then out = O_d Wvu (N x D x E).  Since the
softmax row-normalization is a row scaling, it commutes with the right
multiplication by Wvu, so each core emits the *unnormalized* U = O_d Wvu
plus softmax row-sums and the host combines the two key-halves:
out = (U_even + U_odd) / (s_even + s_odd).

Sharding: core = (batch, key-parity).  Each of the 4 batches is handled by a
pair of cores; core parity c owns the interleaved global key blocks {2j+c}
(128 rows each), which balances the causal work exactly and keeps the
program identical across all 8 cores (masks differ only as data).

In-kernel layout: scores are computed transposed, ST[k, q] = K Q^T, so that
(a) softmax sums over k are a ones-vector matmul, (b) the exp'd tile P[k, q]
is directly the moving operand of O_dT[d, q] += Vd[k, d]^T P (Vd block
stationary), and (c) O_dT column slices are directly the stationary operand
of U = O_d Wvu.  No transposes on-device at all.
"""

import sys

sys.path.insert(0, "/opt/trn_rl_repo")

import numpy as np

import concourse.bacc as bacc
import concourse.mybir as mybir
import concourse.bass_isa as bass_isa
import concourse.tile as tile

F32 = mybir.dt.float32
F32R = mybir.dt.float32r
BF16 = mybir.dt.bfloat16

B, N, E, D = 4, 4096, 1024, 256
NCORES = 8
KLOC = N // 2  # local keys per core
NKB = KLOC // 128  # 16 local key blocks
NQC = N // 512  # 8 query chunks of 512
NKC = KLOC // 512  # 4 local key chunks of 512
SCALE = 1.0 / np.sqrt(np.float32(D))  # 1/16

_CACHE = {}


def _build_nc(reps=1):
    nc = bacc.Bacc("TRN2", target_bir_lowering=False)

    xT = nc.dram_tensor("xT", [E, N], F32R, kind="ExternalInput")
    xkT = nc.dram_tensor("xkT", [E, KLOC], F32R, kind="ExternalInput")
    wq = nc.dram_tensor("wq", [E, D], F32R, kind="ExternalInput")
    wk = nc.dram_tensor("wk", [E, D], F32R, kind="ExternalInput")
    wvd = nc.dram_tensor("wvd", [E, D], F32R, kind="ExternalInput")
    wvu = nc.dram_tensor("wvu", [D, E], F32R, kind="ExternalInput")
    mka = nc.dram_tensor("mka", [128, 512], BF16, kind="ExternalInput")
    mkb = nc.dram_tensor("mkb", [128, 512], BF16, kind="ExternalInput")

    o = nc.dram_tensor("o", [N, E], F32, kind="ExternalOutput")
    ssum = nc.dram_tensor("ssum", [NQC, 512], F32, kind="ExternalOutput")

    with tile.TileContext(nc) as tc:
      for _rep in range(reps):
        with (
            tc.tile_pool(name=f"res{_rep}", bufs=1) as res,
            tc.tile_pool(name=f"consts{_rep}", bufs=1) as consts,
            tc.tile_pool(name=f"pz{_rep}", bufs=1, space="PSUM") as pz,
        ):
            # Resident results of the projection phase.
            qt = [res.tile([128, N], F32R, tag=f"qt{d}", name=f"qt{d}") for d in range(2)]
            kt = [res.tile([128, KLOC], F32R, tag=f"kt{d}", name=f"kt{d}") for d in range(2)]
            vdk = [res.tile([128, D], BF16, tag=f"vdk{kb}", name=f"vdk{kb}") for kb in range(NKB)]

            mask_a = consts.tile([128, 512], BF16, tag="mka")
            mask_b = consts.tile([128, 512], BF16, tag="mkb")
            wvu_t = [consts.tile([128, E], F32R, tag=f"wvu{d}", name=f"wvu{d}") for d in range(2)]

            # The pz PSUM pool spans both phases; per-tag bufs sum to exactly
            # 8 banks (ps 2 + st 4 [shared with proj-vdk] + od 2) so the attention
            # phase can start while late projections still stream (the U
            # matmuls reuse the "ps" tag).

            # ---------------- projections ----------------
            with (
                tc.tile_pool(name="wpool", bufs=1) as wp,
                tc.tile_pool(name="xstream", bufs=2) as xs,
            ):
                wq_t = [wp.tile([128, D], F32R, tag=f"wq{c}", name=f"wq{c}") for c in range(8)]
                wk_t = [wp.tile([128, D], F32R, tag=f"wk{c}", name=f"wk{c}") for c in range(8)]
                wvd_t = [wp.tile([128, D], F32R, tag=f"wvd{c}", name=f"wvd{c}") for c in range(8)]
                for c in range(8):
                    sl = slice(c * 128, (c + 1) * 128)
                    nc.gpsimd.dma_start(out=wk_t[c], in_=wk[sl, :])
                for c in range(8):
                    sl = slice(c * 128, (c + 1) * 128)
                    nc.gpsimd.dma_start(out=wvd_t[c], in_=wvd[sl, :])
                for c in range(8):
                    sl = slice(c * 128, (c + 1) * 128)
                    nc.gpsimd.dma_start(out=wq_t[c], in_=wq[sl, :])
                for d in range(2):
                    nc.gpsimd.dma_start(
                        out=wvu_t[d], in_=wvu[d * 128 : (d + 1) * 128, :]
                    )

                # Merged streaming loop: iteration i does KT / Vd-blocks for
                # local key chunk kc=i (first 4 iterations) and QT for query
                # chunk qc=i.  Each 2MB x-tile is loaded as two halves split
                # across the two HWDGE queues (sync + scalar) so DMA stays
                # ahead of the PE everywhere.
                for i in range(NQC):
                    if i < NKC:
                        xk_h = []
                        for h, eng in ((0, nc.scalar), (1, nc.sync)):
                            xkh = xs.tile(
                                [128, 4, 512], F32R, tag="xk", bufs=3, name=f"xk{h}"
                            )
                            if i == 0:
                                # Quarter-granularity so the very first
                                # matmuls can start after ~256KB, not 1MB.
                                # Sync takes the first half of each 512-row
                                # half (consumed first); scalar, which sits
                                # behind LoadActFuncSet, takes the rest.
                                for cc in range(4):
                                    e2 = nc.sync if cc < 2 else nc.scalar
                                    e2.dma_start(
                                        out=xkh[:, cc, :],
                                        in_=xkT[
                                            h * 512 + cc * 128 : h * 512 + (cc + 1) * 128,
                                            i * 512 : (i + 1) * 512,
                                        ],
                                    )
                            else:
                                eng.dma_start(
                                    out=xkh,
                                    in_=xkT[
                                        h * 512 : (h + 1) * 512, i * 512 : (i + 1) * 512
                                    ].rearrange("(c p) q -> p c q", p=128),
                                )
                            xk_h.append(xkh)
                    if i == 0:
                        # masks ride the sync HWDGE queue right behind the
                        # first xk quarters; still well before attention.
                        nc.sync.dma_start(out=mask_a, in_=mka[:, :])
                        nc.sync.dma_start(out=mask_b, in_=mkb[:, :])
                    xq_h = []
                    for h, eng in ((0, nc.sync), (1, nc.scalar)):
                        xqh = xs.tile(
                            [128, 4, 512], F32R, tag="xq", bufs=5, name=f"xq{h}"
                        )
                        eng.dma_start(
                            out=xqh,
                            in_=xT[
                                h * 512 : (h + 1) * 512, i * 512 : (i + 1) * 512
                            ].rearrange("(c p) q -> p c q", p=128),
                        )
                        xq_h.append(xqh)

                    if i < NKC:
                        # KT chunk
                        for d in range(2):
                            ps = pz.tile([128, 512], F32, tag="ps", bufs=2, name="ps")
                            dsl = slice(d * 128, (d + 1) * 128)
                            for c in range(8):
                                nc.tensor.matmul(
                                    ps,
                                    lhsT=(wk_t[c][:, dsl]),
                                    rhs=(xk_h[c // 4][:, c % 4, :]),
                                    start=(c == 0),
                                    stop=(c == 7),
                                )
                            nc.vector.tensor_copy(
                                kt[d][:, i * 512 : (i + 1) * 512], ps
                            )
                        # Vd blocks, k-major (x block stationary)
                        for j in range(4):
                            kb = 4 * i + j
                            psv = pz.tile([128, D], F32, tag="st", bufs=4, name="psv")
                            for c in range(8):
                                nc.tensor.matmul(
                                    psv,
                                    lhsT=(
                                        xk_h[c // 4][:, c % 4, j * 128 : (j + 1) * 128]
                                    ),
                                    rhs=(wvd_t[c]),
                                    start=(c == 0),
                                    stop=(c == 7),
                                )
                            nc.vector.tensor_copy(vdk[kb], psv)

                    for d in range(2):
                        ps = pz.tile([128, 512], F32, tag="ps", bufs=2, name="ps")
                        dsl = slice(d * 128, (d + 1) * 128)
                        for c in range(8):
                            nc.tensor.matmul(
                                ps,
                                lhsT=(wq_t[c][:, dsl]),
                                rhs=(xq_h[c // 4][:, c % 4, :]),
                                start=(c == 0),
                                stop=(c == 7),
                            )
                        nc.vector.tensor_copy(qt[d][:, i * 512 : (i + 1) * 512], ps)

            # ---------------- attention ----------------
            with (
                tc.tile_pool(name="ppool", bufs=8) as ppool,
                tc.tile_pool(name="stage", bufs=3) as stage,
                tc.tile_pool(name="odstage", bufs=4) as odstage,
                tc.tile_pool(name="sstage", bufs=2) as sstage,
            ):
                for qc in range(NQC):
                    nb = 2 * qc + 2  # local key blocks this query chunk attends to
                    qsl = slice(qc * 512, (qc + 1) * 512)

                    od_ps = pz.tile([128, 2, 512], F32, tag="od", bufs=1, name="od")

                    # 2-deep software pipeline: scores(kb) issue while
                    # od(kb-2) consumes, so the PE never waits on the
                    # ScalarE exp latency.
                    pts = {}
                    levels = [[] for _ in range(6)]

                    def tree_push(t, lv=0):
                        levels[lv].append(t)
                        while len(levels[lv]) >= 2:
                            a = levels[lv].pop(0)
                            b = levels[lv].pop(0)
                            tmp = sstage.tile([128, 512], BF16, tag="tadd", bufs=6, name="tadd")
                            nc.vector.tensor_add(tmp, a, b)
                            lv += 1
                            levels[lv].append(tmp)

                    def emit_scores(kb):
                        ksl = slice(kb * 128, (kb + 1) * 128)
                        # The final (b) diagonal block only attends from
                        # queries >= 256 within the chunk for either parity;
                        # compute just the right half and zero the rest.
                        half = kb == nb - 1
                        csl = slice(256, 512) if half else slice(0, 512)
                        st = pz.tile([128, 512], F32, tag="st", bufs=4, name="st")
                        for d in range(2):
                            nc.tensor.matmul(
                                st[:, csl],
                                lhsT=(kt[d][:, ksl]),
                                rhs=(qt[d][:, qsl][:, csl]),
                                start=(d == 0),
                                stop=(d == 1),
                            )
                        pt = ppool.tile([128, 512], BF16, tag="pt", name="pt")
                        if half:
                            nc.vector.memset(pt[:, 0:256], 0.0)
                        nc.scalar.activation(
                            pt[:, csl], st[:, csl],
                            mybir.ActivationFunctionType.Exp, scale=float(SCALE)
                        )
                        # The last two blocks straddle the causal diagonal.
                        if kb == nb - 2:
                            nc.vector.tensor_mul(pt, pt, mask_a)
                        elif kb == nb - 1:
                            nc.vector.tensor_mul(
                                pt[:, csl], pt[:, csl], mask_b[:, csl]
                            )
                        pts[kb] = pt

                    def emit_consume(kb):
                        pt = pts.pop(kb)
                        half = kb == nb - 1
                        csl = slice(256, 512) if half else slice(0, 512)
                        # O_dT[d, q] += Vd[k, d]^T P[k, q]
                        for d in range(2):
                            nc.tensor.matmul(
                                od_ps[:, d, csl],
                                lhsT=(vdk[kb][:, d * 128 : (d + 1) * 128]),
                                rhs=(pt[:, csl]),
                                start=(kb == 0),
                                stop=(kb == nb - 1),
                            )
                        # softmax denominator partial: pairwise bf16 tree
                        tree_push(pt)

                    for kb in range(nb):
                        emit_scores(kb)
                        if kb >= 2:
                            emit_consume(kb - 2)
                    emit_consume(nb - 2)
                    emit_consume(nb - 1)

                    # fold tree leftovers, then cross-partition reduce on GpSimd
                    rem = [t for lv in levels for t in lv]
                    while len(rem) > 1:
                        tmp = sstage.tile([128, 512], BF16, tag="tadd", bufs=6, name="tadd")
                        nc.vector.tensor_add(tmp, rem.pop(0), rem.pop(0))
                        rem.append(tmp)
                    sred = sstage.tile([128, 512], F32, tag="sred")
                    nc.gpsimd.partition_all_reduce(
                        sred, rem[0], channels=128, reduce_op=bass_isa.ReduceOp.add
                    )
                    nc.gpsimd.dma_start(out=ssum[qc : qc + 1, :], in_=sred[0:1, :])

                    odq = [odstage.tile([128, 512], F32R, tag=f"odq{d}", name=f"odq{d}") for d in range(2)]
                    for d in range(2):
                        nc.vector.tensor_copy(odq[d], od_ps[:, d, :])

                    # U[q, e] = O_dT[:, q]^T Wvu  (unnormalized)
                    for q4 in range(4):
                        qb = qc * 4 + q4
                        q128 = slice(q4 * 128, (q4 + 1) * 128)
                        for eh in range(2):
                            esl = slice(eh * 512, (eh + 1) * 512)
                            ups = pz.tile([128, 512], F32, tag="ps", bufs=2, name="ups")
                            for d in range(2):
                                nc.tensor.matmul(
                                    ups,
                                    lhsT=(odq[d][:, q128]),
                                    rhs=(wvu_t[d][:, esl]),
                                    start=(d == 0),
                                    stop=(d == 1),
                                )
                            ob = stage.tile([128, 512], F32, tag="ob")
                            nc.vector.tensor_copy(ob, ups)
                            oeng = nc.sync if (q4 + eh) % 2 == 0 else nc.gpsimd
                            oeng.dma_start(
                                out=o[qb * 128 : (qb + 1) * 128, esl], in_=ob
                            )
    nc.finalize()
    return nc


def _get_nc():
    if "nc" not in _CACHE:
        _CACHE["nc"] = _build_nc()
    return _CACHE["nc"]


def _host_masks(parity: int):
    y = np.arange(512)[None, :]
    x = np.arange(128)[:, None]
    import ml_dtypes

    mask_a = (y - x - 128 * parity >= 0).astype(ml_dtypes.bfloat16)
    mask_b = (y - x - 256 - 128 * parity >= 0).astype(ml_dtypes.bfloat16)
    return mask_a, mask_b


def _make_in_maps(inputs, Wq, Wk, Wvdown, Wvup):
    inputs = np.asarray(inputs, dtype=np.float32)
    Wq = np.ascontiguousarray(np.asarray(Wq, dtype=np.float32))
    Wk = np.ascontiguousarray(np.asarray(Wk, dtype=np.float32))
    Wvdown = np.ascontiguousarray(np.asarray(Wvdown, dtype=np.float32))
    Wvup = np.ascontiguousarray(np.asarray(Wvup, dtype=np.float32))

    in_maps = []
    for core in range(NCORES):
        b, parity = core // 2, core % 2
        xb = inputs[b]  # [N, E]
        xT = np.ascontiguousarray(xb.T)  # [E, N]
        xk = np.ascontiguousarray(
            xb.reshape(N // 128, 128, E)[parity::2].reshape(KLOC, E)
        )
        xkT = np.ascontiguousarray(xk.T)  # [E, KLOC]
        mask_a, mask_b = _host_masks(parity)
        in_maps.append(
            {
                "xT": xT,
                "xkT": xkT,
                "wq": Wq,
                "wk": Wk,
                "wvd": Wvdown,
                "wvu": Wvup,
                "mka": mask_a,
                "mkb": mask_b,
            }
        )
    return in_maps


def _assemble(results):
    out = np.empty((B, N, E), dtype=np.float32)
    for b in range(B):
        o_sum = results[2 * b]["o"] + results[2 * b + 1]["o"]
        s_sum = (results[2 * b]["ssum"] + results[2 * b + 1]["ssum"]).reshape(N)
        out[b] = o_sum / s_sum[:, None]
    return out


def kernel(inputs, Wq, Wk, Wvdown, Wvup):
    from concourse.bass_utils import run_bass_kernel_spmd

    nc = _get_nc()
    in_maps = _make_in_maps(inputs, Wq, Wk, Wvdown, Wvup)

    res = run_bass_kernel_spmd(nc, in_maps, core_ids=list(range(NCORES)))
    return _assemble(res.results)


# revision 37
# speedup vs baseline: 1.0017x; 1.0017x over previous
"""Causal dot-product attention (low-rank V) on 8 Trainium2 NeuronCores.

Problem: inputs [B=4, N=4096, E=1024], Wq/Wk/Wvdown [E, D=256], Wvup [D, E].
    Q = x Wq; K = x Wk; S = Q K^T / sqrt(D) (causal); A = softmax(S)
    V = x Wvdown Wvup; out = A V

Key algebraic win vs the naive schedule: V has rank <= D=256, so instead of
out = A V with V = x Wvd Wvu (an N x N x E contraction), compute
O_d = A (x Wvd) (N x N x D) and then out = O_d Wvu (N x D x E).  Since the
softmax row-normalization is a row scaling, it commutes with the right
multiplication by Wvu, so each core emits the *unnormalized* U = O_d Wvu
plus softmax row-sums and the host combines the two key-halves:
out = (U_even + U_odd) / (s_even + s_odd).

Sharding: core = (batch, key-parity).  Each of the 4 batches is handled by a
pair of cores; core parity c owns the interleaved global key blocks {2j+c}
(128 rows each), which balances the causal work exactly and keeps the
program identical across all 8 cores (masks differ only as data).

In-kernel layout: scores are computed transposed, ST[k, q] = K Q^T, so that
(a) softmax sums over k are a ones-vector matmul, (b) the exp'd tile P[k, q]
is directly the moving operand of O_dT[d, q] += Vd[k, d]^T P (Vd block
stationary), and (c) O_dT column slices are directly the stationary operand
of U = O_d Wvu.  No transposes on-device at all.
"""

import sys

sys.path.insert(0, "/opt/trn_rl_repo")

import numpy as np

import concourse.bacc as bacc
import concourse.mybir as mybir
import concourse.bass_isa as bass_isa
import concourse.tile as tile

F32 = mybir.dt.float32
F32R = mybir.dt.float32r
BF16 = mybir.dt.bfloat16

B, N, E, D = 4, 4096, 1024, 256
NCORES = 8
KLOC = N // 2  # local keys per core
NKB = KLOC // 128  # 16 local key blocks
NQC = N // 512  # 8 query chunks of 512
NKC = KLOC // 512  # 4 local key chunks of 512
SCALE = 1.0 / np.sqrt(np.float32(D))  # 1/16

_CACHE = {}


def _build_nc(reps=1):
    nc = bacc.Bacc("TRN2", target_bir_lowering=False)

    xT = nc.dram_tensor("xT", [E, N], F32R, kind="ExternalInput")
    xkT = nc.dram_tensor("xkT", [E, KLOC], F32R, kind="ExternalInput")
    wq = nc.dram_tensor("wq", [E, D], F32R, kind="ExternalInput")
    wk = nc.dram_tensor("wk", [E, D], F32R, kind="ExternalInput")
    wvd = nc.dram_tensor("wvd", [E, D], F32R, kind="ExternalInput")
    wvu = nc.dram_tensor("wvu", [D, E], F32R, kind="ExternalInput")
    mka = nc.dram_tensor("mka", [128, 512], BF16, kind="ExternalInput")
    mkb = nc.dram_tensor("mkb", [128, 512], BF16, kind="ExternalInput")

    o = nc.dram_tensor("o", [N, E], F32, kind="ExternalOutput")
    ssum = nc.dram_tensor("ssum", [NQC, 512], F32, kind="ExternalOutput")

    with tile.TileContext(nc) as tc:
      for _rep in range(reps):
        with (
            tc.tile_pool(name=f"res{_rep}", bufs=1) as res,
            tc.tile_pool(name=f"consts{_rep}", bufs=1) as consts,
            tc.tile_pool(name=f"pz{_rep}", bufs=1, space="PSUM") as pz,
        ):
            # Resident results of the projection phase.
            qt = [res.tile([128, N], F32R, tag=f"qt{d}", name=f"qt{d}") for d in range(2)]
            kt = [res.tile([128, KLOC], F32R, tag=f"kt{d}", name=f"kt{d}") for d in range(2)]
            vdk = [res.tile([128, D], BF16, tag=f"vdk{kb}", name=f"vdk{kb}") for kb in range(NKB)]

            mask_a = consts.tile([128, 512], BF16, tag="mka")
            mask_b = consts.tile([128, 512], BF16, tag="mkb")
            wvu_t = [consts.tile([128, E], F32R, tag=f"wvu{d}", name=f"wvu{d}") for d in range(2)]

            # The pz PSUM pool spans both phases; per-tag bufs sum to exactly
            # 8 banks (ps 2 + st 4 [shared with proj-vdk] + od 2) so the attention
            # phase can start while late projections still stream (the U
            # matmuls reuse the "ps" tag).

            # ---------------- projections ----------------
            with (
                tc.tile_pool(name="wpool", bufs=1) as wp,
                tc.tile_pool(name="xstream", bufs=2) as xs,
            ):
                wq_t = [wp.tile([128, D], F32R, tag=f"wq{c}", name=f"wq{c}") for c in range(8)]
                wk_t = [wp.tile([128, D], F32R, tag=f"wk{c}", name=f"wk{c}") for c in range(8)]
                wvd_t = [wp.tile([128, D], F32R, tag=f"wvd{c}", name=f"wvd{c}") for c in range(8)]
                for c in range(8):
                    sl = slice(c * 128, (c + 1) * 128)
                    nc.gpsimd.dma_start(out=wk_t[c], in_=wk[sl, :])
                for c in range(8):
                    sl = slice(c * 128, (c + 1) * 128)
                    nc.gpsimd.dma_start(out=wvd_t[c], in_=wvd[sl, :])
                for c in range(8):
                    sl = slice(c * 128, (c + 1) * 128)
                    nc.gpsimd.dma_start(out=wq_t[c], in_=wq[sl, :])
                for d in range(2):
                    nc.gpsimd.dma_start(
                        out=wvu_t[d], in_=wvu[d * 128 : (d + 1) * 128, :]
                    )

                # Merged streaming loop: iteration i does KT / Vd-blocks for
                # local key chunk kc=i (first 4 iterations) and QT for query
                # chunk qc=i.  Each 2MB x-tile is loaded as two halves split
                # across the two HWDGE queues (sync + scalar) so DMA stays
                # ahead of the PE everywhere.
                for i in range(NQC):
                    if i < NKC:
                        xk_h = []
                        for h, eng in ((0, nc.scalar), (1, nc.sync)):
                            xkh = xs.tile(
                                [128, 4, 512], F32R, tag="xk", bufs=3, name=f"xk{h}"
                            )
                            if i == 0:
                                # Quarter-granularity so the very first
                                # matmuls can start after ~256KB, not 1MB.
                                # Sync takes the first half of each 512-row
                                # half (consumed first); scalar, which sits
                                # behind LoadActFuncSet, takes the rest.
                                for cc in range(4):
                                    e2 = nc.sync if cc < 2 else nc.scalar
                                    e2.dma_start(
                                        out=xkh[:, cc, :],
                                        in_=xkT[
                                            h * 512 + cc * 128 : h * 512 + (cc + 1) * 128,
                                            i * 512 : (i + 1) * 512,
                                        ],
                                    )
                            else:
                                eng.dma_start(
                                    out=xkh,
                                    in_=xkT[
                                        h * 512 : (h + 1) * 512, i * 512 : (i + 1) * 512
                                    ].rearrange("(c p) q -> p c q", p=128),
                                )
                            xk_h.append(xkh)
                    if i == 0:
                        # masks ride the sync HWDGE queue right behind the
                        # first xk quarters; still well before attention.
                        nc.sync.dma_start(out=mask_a, in_=mka[:, :])
                        nc.sync.dma_start(out=mask_b, in_=mkb[:, :])
                    xq_h = []
                    for h, eng in ((0, nc.sync), (1, nc.scalar)):
                        xqh = xs.tile(
                            [128, 4, 512], F32R, tag="xq", bufs=5, name=f"xq{h}"
                        )
                        eng.dma_start(
                            out=xqh,
                            in_=xT[
                                h * 512 : (h + 1) * 512, i * 512 : (i + 1) * 512
                            ].rearrange("(c p) q -> p c q", p=128),
                        )
                        xq_h.append(xqh)

                    if i < NKC:
                        # KT chunk
                        for d in range(2):
                            ps = pz.tile([128, 512], F32, tag="ps", bufs=2, name="ps")
                            dsl = slice(d * 128, (d + 1) * 128)
                            for c in range(8):
                                nc.tensor.matmul(
                                    ps,
                                    lhsT=(wk_t[c][:, dsl]),
                                    rhs=(xk_h[c // 4][:, c % 4, :]),
                                    start=(c == 0),
                                    stop=(c == 7),
                                )
                            nc.vector.tensor_copy(
                                kt[d][:, i * 512 : (i + 1) * 512], ps
                            )
                        # Vd blocks, k-major (x block stationary)
                        for j in range(4):
                            kb = 4 * i + j
                            psv = pz.tile([128, D], F32, tag="st", bufs=4, name="psv")
                            for c in range(8):
                                nc.tensor.matmul(
                                    psv,
                                    lhsT=(
                                        xk_h[c // 4][:, c % 4, j * 128 : (j + 1) * 128]
                                    ),
                                    rhs=(wvd_t[c]),
                                    start=(c == 0),
                                    stop=(c == 7),
                                )
                            nc.vector.tensor_copy(vdk[kb], psv)

                    for d in range(2):
                        ps = pz.tile([128, 512], F32, tag="ps", bufs=2, name="ps")
                        dsl = slice(d * 128, (d + 1) * 128)
                        for c in range(8):
                            nc.tensor.matmul(
                                ps,
                                lhsT=(wq_t[c][:, dsl]),
                                rhs=(xq_h[c // 4][:, c % 4, :]),
                                start=(c == 0),
                                stop=(c == 7),
                            )
                        nc.vector.tensor_copy(qt[d][:, i * 512 : (i + 1) * 512], ps)

            # ---------------- attention ----------------
            with (
                tc.tile_pool(name="ppool", bufs=8) as ppool,
                tc.tile_pool(name="stage", bufs=3) as stage,
                tc.tile_pool(name="odstage", bufs=4) as odstage,
                tc.tile_pool(name="sstage", bufs=2) as sstage,
            ):
                for qc in range(NQC):
                    nb = 2 * qc + 2  # local key blocks this query chunk attends to
                    qsl = slice(qc * 512, (qc + 1) * 512)

                    od_ps = pz.tile([128, 2, 512], F32, tag="od", bufs=1, name="od")

                    # 2-deep software pipeline: scores(kb) issue while
                    # od(kb-2) consumes, so the PE never waits on the
                    # ScalarE exp latency.
                    pts = {}
                    levels = [[] for _ in range(6)]

                    def tree_push(t, lv=0):
                        levels[lv].append(t)
                        while len(levels[lv]) >= 2:
                            a = levels[lv].pop(0)
                            b = levels[lv].pop(0)
                            tmp = sstage.tile([128, 512], BF16, tag="tadd", bufs=6, name="tadd")
                            # last chunk: adds ride GpSimd so DVE is free for
                            # the kernel-tail psum copies
                            eng = nc.gpsimd if qc == NQC - 1 else nc.vector
                            eng.tensor_add(tmp, a, b)
                            lv += 1
                            levels[lv].append(tmp)

                    def emit_scores(kb):
                        ksl = slice(kb * 128, (kb + 1) * 128)
                        # The final (b) diagonal block only attends from
                        # queries >= 256 within the chunk for either parity;
                        # compute just the right half and zero the rest.
                        half = kb == nb - 1
                        csl = slice(256, 512) if half else slice(0, 512)
                        st = pz.tile([128, 512], F32, tag="st", bufs=4, name="st")
                        for d in range(2):
                            nc.tensor.matmul(
                                st[:, csl],
                                lhsT=(kt[d][:, ksl]),
                                rhs=(qt[d][:, qsl][:, csl]),
                                start=(d == 0),
                                stop=(d == 1),
                            )
                        pt = ppool.tile([128, 512], BF16, tag="pt", name="pt")
                        if half:
                            nc.vector.memset(pt[:, 0:256], 0.0)
                        nc.scalar.activation(
                            pt[:, csl], st[:, csl],
                            mybir.ActivationFunctionType.Exp, scale=float(SCALE)
                        )
                        # The last two blocks straddle the causal diagonal.
                        if kb == nb - 2:
                            nc.vector.tensor_mul(pt, pt, mask_a)
                        elif kb == nb - 1:
                            nc.vector.tensor_mul(
                                pt[:, csl], pt[:, csl], mask_b[:, csl]
                            )
                        pts[kb] = pt

                    def emit_consume(kb):
                        pt = pts.pop(kb)
                        half = kb == nb - 1
                        csl = slice(256, 512) if half else slice(0, 512)
                        # O_dT[d, q] += Vd[k, d]^T P[k, q]
                        for d in range(2):
                            nc.tensor.matmul(
                                od_ps[:, d, csl],
                                lhsT=(vdk[kb][:, d * 128 : (d + 1) * 128]),
                                rhs=(pt[:, csl]),
                                start=(kb == 0),
                                stop=(kb == nb - 1),
                            )
                        # softmax denominator partial: pairwise bf16 tree
                        tree_push(pt)

                    for kb in range(nb):
                        emit_scores(kb)
                        if kb >= 2:
                            emit_consume(kb - 2)
                    emit_consume(nb - 2)
                    emit_consume(nb - 1)

                    # fold tree leftovers, then cross-partition reduce on GpSimd
                    rem = [t for lv in levels for t in lv]
                    while len(rem) > 1:
                        tmp = sstage.tile([128, 512], BF16, tag="tadd", bufs=6, name="tadd")
                        eng = nc.gpsimd if qc == NQC - 1 else nc.vector
                        eng.tensor_add(tmp, rem.pop(0), rem.pop(0))
                        rem.append(tmp)
                    sred = sstage.tile([128, 512], F32, tag="sred")
                    nc.gpsimd.partition_all_reduce(
                        sred, rem[0], channels=128, reduce_op=bass_isa.ReduceOp.add
                    )
                    nc.gpsimd.dma_start(out=ssum[qc : qc + 1, :], in_=sred[0:1, :])

                    odq = [odstage.tile([128, 512], F32R, tag=f"odq{d}", name=f"odq{d}") for d in range(2)]
                    for d in range(2):
                        nc.vector.tensor_copy(odq[d], od_ps[:, d, :])

                    # U[q, e] = O_dT[:, q]^T Wvu  (unnormalized)
                    for q4 in range(4):
                        qb = qc * 4 + q4
                        q128 = slice(q4 * 128, (q4 + 1) * 128)
                        for eh in range(2):
                            esl = slice(eh * 512, (eh + 1) * 512)
                            ups = pz.tile([128, 512], F32, tag="ps", bufs=2, name="ups")
                            for d in range(2):
                                nc.tensor.matmul(
                                    ups,
                                    lhsT=(odq[d][:, q128]),
                                    rhs=(wvu_t[d][:, esl]),
                                    start=(d == 0),
                                    stop=(d == 1),
                                )
                            ob = stage.tile([128, 512], F32, tag="ob")
                            nc.vector.tensor_copy(ob, ups)
                            oeng = nc.sync if (q4 + eh) % 2 == 0 else nc.gpsimd
                            oeng.dma_start(
                                out=o[qb * 128 : (qb + 1) * 128, esl], in_=ob
                            )
    nc.finalize()
    return nc


def _get_nc():
    if "nc" not in _CACHE:
        _CACHE["nc"] = _build_nc()
    return _CACHE["nc"]


def _host_masks(parity: int):
    y = np.arange(512)[None, :]
    x = np.arange(128)[:, None]
    import ml_dtypes

    mask_a = (y - x - 128 * parity >= 0).astype(ml_dtypes.bfloat16)
    mask_b = (y - x - 256 - 128 * parity >= 0).astype(ml_dtypes.bfloat16)
    return mask_a, mask_b


def _make_in_maps(inputs, Wq, Wk, Wvdown, Wvup):
    inputs = np.asarray(inputs, dtype=np.float32)
    Wq = np.ascontiguousarray(np.asarray(Wq, dtype=np.float32))
    Wk = np.ascontiguousarray(np.asarray(Wk, dtype=np.float32))
    Wvdown = np.ascontiguousarray(np.asarray(Wvdown, dtype=np.float32))
    Wvup = np.ascontiguousarray(np.asarray(Wvup, dtype=np.float32))

    in_maps = []
    for core in range(NCORES):
        b, parity = core // 2, core % 2
        xb = inputs[b]  # [N, E]
        xT = np.ascontiguousarray(xb.T)  # [E, N]
        xk = np.ascontiguousarray(
            xb.reshape(N // 128, 128, E)[parity::2].reshape(KLOC, E)
        )
        xkT = np.ascontiguousarray(xk.T)  # [E, KLOC]
        mask_a, mask_b = _host_masks(parity)
        in_maps.append(
            {
                "xT": xT,
                "xkT": xkT,
                "wq": Wq,
                "wk": Wk,
                "wvd": Wvdown,
                "wvu": Wvup,
                "mka": mask_a,
                "mkb": mask_b,
            }
        )
    return in_maps


def _assemble(results):
    out = np.empty((B, N, E), dtype=np.float32)
    for b in range(B):
        o_sum = results[2 * b]["o"] + results[2 * b + 1]["o"]
        s_sum = (results[2 * b]["ssum"] + results[2 * b + 1]["ssum"]).reshape(N)
        out[b] = o_sum / s_sum[:, None]
    return out


def kernel(inputs, Wq, Wk, Wvdown, Wvup):
    from concourse.bass_utils import run_bass_kernel_spmd

    nc = _get_nc()
    in_maps = _make_in_maps(inputs, Wq, Wk, Wvdown, Wvup)

    res = run_bass_kernel_spmd(nc, in_maps, core_ids=list(range(NCORES)))
    return _assemble(res.results)


# revision 41
# speedup vs baseline: 1.0028x; 1.0012x over previous
"""Causal dot-product attention (low-rank V) on 8 Trainium2 NeuronCores.

Problem: inputs [B=4, N=4096, E=1024], Wq/Wk/Wvdown [E, D=256], Wvup [D, E].
    Q = x Wq; K = x Wk; S = Q K^T / sqrt(D) (causal); A = softmax(S)
    V = x Wvdown Wvup; out = A V

Key algebraic win vs the naive schedule: V has rank <= D=256, so instead of
out = A V with V = x Wvd Wvu (an N x N x E contraction), compute
O_d = A (x Wvd) (N x N x D) and then out = O_d Wvu (N x D x E).  Since the
softmax row-normalization is a row scaling, it commutes with the right
multiplication by Wvu, so each core emits the *unnormalized* U = O_d Wvu
plus softmax row-sums and the host combines the two key-halves:
out = (U_even + U_odd) / (s_even + s_odd).

Sharding: core = (batch, key-parity).  Each of the 4 batches is handled by a
pair of cores; core parity c owns the interleaved global key blocks {2j+c}
(128 rows each), which balances the causal work exactly and keeps the
program identical across all 8 cores (masks differ only as data).

In-kernel layout: scores are computed transposed, ST[k, q] = K Q^T, so that
(a) softmax sums over k are a ones-vector matmul, (b) the exp'd tile P[k, q]
is directly the moving operand of O_dT[d, q] += Vd[k, d]^T P (Vd block
stationary), and (c) O_dT column slices are directly the stationary operand
of U = O_d Wvu.  No transposes on-device at all.
"""

import sys

sys.path.insert(0, "/opt/trn_rl_repo")

import numpy as np

import concourse.bacc as bacc
import concourse.mybir as mybir
import concourse.bass_isa as bass_isa
import concourse.tile as tile

F32 = mybir.dt.float32
F32R = mybir.dt.float32r
BF16 = mybir.dt.bfloat16

B, N, E, D = 4, 4096, 1024, 256
NCORES = 8
KLOC = N // 2  # local keys per core
NKB = KLOC // 128  # 16 local key blocks
NQC = N // 512  # 8 query chunks of 512
NKC = KLOC // 512  # 4 local key chunks of 512
SCALE = 1.0 / np.sqrt(np.float32(D))  # 1/16

_CACHE = {}


def _build_nc(reps=1):
    nc = bacc.Bacc("TRN2", target_bir_lowering=False)

    xT = nc.dram_tensor("xT", [E, N], F32R, kind="ExternalInput")
    xkT = nc.dram_tensor("xkT", [E, KLOC], F32R, kind="ExternalInput")
    wq = nc.dram_tensor("wq", [E, D], F32R, kind="ExternalInput")
    wk = nc.dram_tensor("wk", [E, D], F32R, kind="ExternalInput")
    wvd = nc.dram_tensor("wvd", [E, D], F32R, kind="ExternalInput")
    wvu = nc.dram_tensor("wvu", [D, E], F32R, kind="ExternalInput")
    mka = nc.dram_tensor("mka", [128, 512], BF16, kind="ExternalInput")
    mkb = nc.dram_tensor("mkb", [128, 512], BF16, kind="ExternalInput")

    o = nc.dram_tensor("o", [N, E], F32, kind="ExternalOutput")
    ssum = nc.dram_tensor("ssum", [NQC, 512], F32, kind="ExternalOutput")

    with tile.TileContext(nc) as tc:
      for _rep in range(reps):
        with (
            tc.tile_pool(name=f"res{_rep}", bufs=1) as res,
            tc.tile_pool(name=f"consts{_rep}", bufs=1) as consts,
            tc.tile_pool(name=f"pz{_rep}", bufs=1, space="PSUM") as pz,
        ):
            # Resident results of the projection phase.
            qt = [res.tile([128, N], F32R, tag=f"qt{d}", name=f"qt{d}") for d in range(2)]
            kt = [res.tile([128, KLOC], F32R, tag=f"kt{d}", name=f"kt{d}") for d in range(2)]
            vdk = [res.tile([128, D], BF16, tag=f"vdk{kb}", name=f"vdk{kb}") for kb in range(NKB)]

            mask_a = consts.tile([128, 512], BF16, tag="mka")
            mask_b = consts.tile([128, 512], BF16, tag="mkb")
            wvu_t = [consts.tile([128, E], F32R, tag=f"wvu{d}", name=f"wvu{d}") for d in range(2)]

            # The pz PSUM pool spans both phases; per-tag bufs sum to exactly
            # 8 banks (ps 2 + st 4 [shared with proj-vdk] + od 2) so the attention
            # phase can start while late projections still stream (the U
            # matmuls reuse the "ps" tag).

            # ---------------- projections ----------------
            with (
                tc.tile_pool(name="wpool", bufs=1) as wp,
                tc.tile_pool(name="xstream", bufs=2) as xs,
            ):
                wq_t = [wp.tile([128, D], F32R, tag=f"wq{c}", name=f"wq{c}") for c in range(8)]
                wk_t = [wp.tile([128, D], F32R, tag=f"wk{c}", name=f"wk{c}") for c in range(8)]
                wvd_t = [wp.tile([128, D], F32R, tag=f"wvd{c}", name=f"wvd{c}") for c in range(8)]
                for c in range(8):
                    sl = slice(c * 128, (c + 1) * 128)
                    nc.gpsimd.dma_start(out=wk_t[c], in_=wk[sl, :])
                for c in range(8):
                    sl = slice(c * 128, (c + 1) * 128)
                    nc.gpsimd.dma_start(out=wvd_t[c], in_=wvd[sl, :])
                for c in range(8):
                    sl = slice(c * 128, (c + 1) * 128)
                    nc.gpsimd.dma_start(out=wq_t[c], in_=wq[sl, :])
                for d in range(2):
                    nc.gpsimd.dma_start(
                        out=wvu_t[d], in_=wvu[d * 128 : (d + 1) * 128, :]
                    )

                # Merged streaming loop: iteration i does KT / Vd-blocks for
                # local key chunk kc=i (first 4 iterations) and QT for query
                # chunk qc=i.  Each 2MB x-tile is loaded as two halves split
                # across the two HWDGE queues (sync + scalar) so DMA stays
                # ahead of the PE everywhere.
                for i in range(NQC):
                    if i < NKC:
                        xk_h = []
                        for h, eng in ((0, nc.scalar), (1, nc.sync)):
                            xkh = xs.tile(
                                [128, 4, 512], F32R, tag="xk", bufs=3, name=f"xk{h}"
                            )
                            if i == 0:
                                # Quarter-granularity so the very first
                                # matmuls can start after ~256KB, not 1MB.
                                # Sync takes the first half of each 512-row
                                # half (consumed first); scalar, which sits
                                # behind LoadActFuncSet, takes the rest.
                                for cc in range(4):
                                    e2 = nc.sync if cc < 2 else nc.scalar
                                    e2.dma_start(
                                        out=xkh[:, cc, :],
                                        in_=xkT[
                                            h * 512 + cc * 128 : h * 512 + (cc + 1) * 128,
                                            i * 512 : (i + 1) * 512,
                                        ],
                                    )
                            else:
                                eng.dma_start(
                                    out=xkh,
                                    in_=xkT[
                                        h * 512 : (h + 1) * 512, i * 512 : (i + 1) * 512
                                    ].rearrange("(c p) q -> p c q", p=128),
                                )
                            xk_h.append(xkh)
                    if i == 0:
                        # masks ride the sync HWDGE queue right behind the
                        # first xk quarters; still well before attention.
                        nc.sync.dma_start(out=mask_a, in_=mka[:, :])
                        nc.sync.dma_start(out=mask_b, in_=mkb[:, :])
                    xq_h = []
                    for h, eng in ((0, nc.sync), (1, nc.scalar)):
                        xqh = xs.tile(
                            [128, 4, 512], F32R, tag="xq", bufs=5, name=f"xq{h}"
                        )
                        eng.dma_start(
                            out=xqh,
                            in_=xT[
                                h * 512 : (h + 1) * 512, i * 512 : (i + 1) * 512
                            ].rearrange("(c p) q -> p c q", p=128),
                        )
                        xq_h.append(xqh)

                    if i < NKC:
                        # KT chunk
                        for d in range(2):
                            ps = pz.tile([128, 512], F32, tag="ps", bufs=2, name="ps")
                            dsl = slice(d * 128, (d + 1) * 128)
                            for c in range(8):
                                nc.tensor.matmul(
                                    ps,
                                    lhsT=(wk_t[c][:, dsl]),
                                    rhs=(xk_h[c // 4][:, c % 4, :]),
                                    start=(c == 0),
                                    stop=(c == 7),
                                )
                            nc.vector.tensor_copy(
                                kt[d][:, i * 512 : (i + 1) * 512], ps
                            )
                        # Vd blocks, k-major (x block stationary)
                        for j in range(4):
                            kb = 4 * i + j
                            psv = pz.tile([128, D], F32, tag="st", bufs=4, name="psv")
                            for c in range(8):
                                nc.tensor.matmul(
                                    psv,
                                    lhsT=(
                                        xk_h[c // 4][:, c % 4, j * 128 : (j + 1) * 128]
                                    ),
                                    rhs=(wvd_t[c]),
                                    start=(c == 0),
                                    stop=(c == 7),
                                )
                            nc.vector.tensor_copy(vdk[kb], psv)

                    for d in range(2):
                        ps = pz.tile([128, 512], F32, tag="ps", bufs=2, name="ps")
                        dsl = slice(d * 128, (d + 1) * 128)
                        for c in range(8):
                            nc.tensor.matmul(
                                ps,
                                lhsT=(wq_t[c][:, dsl]),
                                rhs=(xq_h[c // 4][:, c % 4, :]),
                                start=(c == 0),
                                stop=(c == 7),
                            )
                        nc.vector.tensor_copy(qt[d][:, i * 512 : (i + 1) * 512], ps)

            # ---------------- attention ----------------
            with (
                tc.tile_pool(name="ppool", bufs=8) as ppool,
                tc.tile_pool(name="stage", bufs=3) as stage,
                tc.tile_pool(name="odstage", bufs=4) as odstage,
                tc.tile_pool(name="sstage", bufs=2) as sstage,
            ):
                for qc in range(NQC):
                    nb = 2 * qc + 2  # local key blocks this query chunk attends to
                    qsl = slice(qc * 512, (qc + 1) * 512)

                    od_ps = pz.tile([128, 2, 512], F32, tag="od", bufs=1, name="od")

                    # 2-deep software pipeline: scores(kb) issue while
                    # od(kb-2) consumes, so the PE never waits on the
                    # ScalarE exp latency.
                    pts = {}
                    levels = [[] for _ in range(6)]

                    def tree_push(t, lv=0):
                        levels[lv].append(t)
                        while len(levels[lv]) >= 2:
                            a = levels[lv].pop(0)
                            b = levels[lv].pop(0)
                            tmp = sstage.tile([128, 512], BF16, tag="tadd", bufs=6, name="tadd")
                            # last chunk: adds ride GpSimd so DVE is free for
                            # the kernel-tail psum copies
                            eng = nc.gpsimd if qc == NQC - 1 else nc.vector
                            eng.tensor_add(tmp, a, b)
                            lv += 1
                            levels[lv].append(tmp)

                    def emit_scores(kb):
                        ksl = slice(kb * 128, (kb + 1) * 128)
                        # The final (b) diagonal block only attends from
                        # queries >= 256 within the chunk for either parity;
                        # compute just the right half and zero the rest.
                        half = kb == nb - 1
                        csl = slice(256, 512) if half else slice(0, 512)
                        st = pz.tile([128, 512], F32, tag="st", bufs=4, name="st")
                        for d in range(2):
                            nc.tensor.matmul(
                                st[:, csl],
                                lhsT=(kt[d][:, ksl]),
                                rhs=(qt[d][:, qsl][:, csl]),
                                start=(d == 0),
                                stop=(d == 1),
                            )
                        pt = ppool.tile([128, 512], BF16, tag="pt", name="pt")
                        if half:
                            nc.vector.memset(pt[:, 0:256], 0.0)
                        nc.scalar.activation(
                            pt[:, csl], st[:, csl],
                            mybir.ActivationFunctionType.Exp, scale=float(SCALE)
                        )
                        # The last two blocks straddle the causal diagonal.
                        if kb == nb - 2:
                            nc.vector.tensor_mul(pt, pt, mask_a)
                        elif kb == nb - 1:
                            nc.vector.tensor_mul(
                                pt[:, csl], pt[:, csl], mask_b[:, csl]
                            )
                        pts[kb] = pt

                    def emit_consume(kb):
                        pt = pts.pop(kb)
                        half = kb == nb - 1
                        csl = slice(256, 512) if half else slice(0, 512)
                        # O_dT[d, q] += Vd[k, d]^T P[k, q]
                        for d in range(2):
                            nc.tensor.matmul(
                                od_ps[:, d, csl],
                                lhsT=(vdk[kb][:, d * 128 : (d + 1) * 128]),
                                rhs=(pt[:, csl]),
                                start=(kb == 0),
                                stop=(kb == nb - 1),
                            )
                        # softmax denominator partial: pairwise bf16 tree
                        tree_push(pt)

                    LAG = 3
                    for kb in range(nb):
                        emit_scores(kb)
                        if kb >= LAG:
                            emit_consume(kb - LAG)
                    for kb in range(max(0, nb - LAG), nb):
                        emit_consume(kb)

                    # fold tree leftovers, then cross-partition reduce on GpSimd
                    rem = [t for lv in levels for t in lv]
                    while len(rem) > 1:
                        tmp = sstage.tile([128, 512], BF16, tag="tadd", bufs=6, name="tadd")
                        eng = nc.gpsimd if qc == NQC - 1 else nc.vector
                        eng.tensor_add(tmp, rem.pop(0), rem.pop(0))
                        rem.append(tmp)
                    sred = sstage.tile([128, 512], F32, tag="sred")
                    nc.gpsimd.partition_all_reduce(
                        sred, rem[0], channels=128, reduce_op=bass_isa.ReduceOp.add
                    )
                    nc.gpsimd.dma_start(out=ssum[qc : qc + 1, :], in_=sred[0:1, :])

                    odq = [odstage.tile([128, 512], F32R, tag=f"odq{d}", name=f"odq{d}") for d in range(2)]
                    for d in range(2):
                        nc.vector.tensor_copy(odq[d], od_ps[:, d, :])

                    # U[q, e] = O_dT[:, q]^T Wvu  (unnormalized)
                    for q4 in range(4):
                        qb = qc * 4 + q4
                        q128 = slice(q4 * 128, (q4 + 1) * 128)
                        for eh in range(2):
                            esl = slice(eh * 512, (eh + 1) * 512)
                            ups = pz.tile([128, 512], F32, tag="ps", bufs=2, name="ups")
                            for d in range(2):
                                nc.tensor.matmul(
                                    ups,
                                    lhsT=(odq[d][:, q128]),
                                    rhs=(wvu_t[d][:, esl]),
                                    start=(d == 0),
                                    stop=(d == 1),
                                )
                            ob = stage.tile([128, 512], F32, tag="ob")
                            nc.vector.tensor_copy(ob, ups)
                            oeng = nc.sync if (q4 + eh) % 2 == 0 else nc.gpsimd
                            oeng.dma_start(
                                out=o[qb * 128 : (qb + 1) * 128, esl], in_=ob
                            )
    nc.finalize()
    return nc


def _get_nc():
    if "nc" not in _CACHE:
        _CACHE["nc"] = _build_nc()
    return _CACHE["nc"]


def _host_masks(parity: int):
    y = np.arange(512)[None, :]
    x = np.arange(128)[:, None]
    import ml_dtypes

    mask_a = (y - x - 128 * parity >= 0).astype(ml_dtypes.bfloat16)
    mask_b = (y - x - 256 - 128 * parity >= 0).astype(ml_dtypes.bfloat16)
    return mask_a, mask_b


def _make_in_maps(inputs, Wq, Wk, Wvdown, Wvup):
    inputs = np.asarray(inputs, dtype=np.float32)
    Wq = np.ascontiguousarray(np.asarray(Wq, dtype=np.float32))
    Wk = np.ascontiguousarray(np.asarray(Wk, dtype=np.float32))
    Wvdown = np.ascontiguousarray(np.asarray(Wvdown, dtype=np.float32))
    Wvup = np.ascontiguousarray(np.asarray(Wvup, dtype=np.float32))

    in_maps = []
    for core in range(NCORES):
        b, parity = core // 2, core % 2
        xb = inputs[b]  # [N, E]
        xT = np.ascontiguousarray(xb.T)  # [E, N]
        xk = np.ascontiguousarray(
            xb.reshape(N // 128, 128, E)[parity::2].reshape(KLOC, E)
        )
        xkT = np.ascontiguousarray(xk.T)  # [E, KLOC]
        mask_a, mask_b = _host_masks(parity)
        in_maps.append(
            {
                "xT": xT,
                "xkT": xkT,
                "wq": Wq,
                "wk": Wk,
                "wvd": Wvdown,
                "wvu": Wvup,
                "mka": mask_a,
                "mkb": mask_b,
            }
        )
    return in_maps


def _assemble(results):
    out = np.empty((B, N, E), dtype=np.float32)
    for b in range(B):
        o_sum = results[2 * b]["o"] + results[2 * b + 1]["o"]
        s_sum = (results[2 * b]["ssum"] + results[2 * b + 1]["ssum"]).reshape(N)
        out[b] = o_sum / s_sum[:, None]
    return out


def kernel(inputs, Wq, Wk, Wvdown, Wvup):
    from concourse.bass_utils import run_bass_kernel_spmd

    nc = _get_nc()
    in_maps = _make_in_maps(inputs, Wq, Wk, Wvdown, Wvup)

    res = run_bass_kernel_spmd(nc, in_maps, core_ids=list(range(NCORES)))
    return _assemble(res.results)
